# revision 3
# baseline (speedup 1.0000x reference)
"""NeighborCooccurrenceEncoder Trainium2 kernel (8-core SPMD, data-parallel over B).

Algorithm per core (512 batch rows):
  1. Equality counting via PE matmul: D[i,j] = ((a_i - c_j)/32)^2 computed
     EXACTLY with fp16 operands (split-square trick) + fp32 PSUM accumulation.
     K=5 matmul: lhsT=(aaH, aaL, sa, 1, 1), rhs=(1, 1, -2sc, ccH, ccL).
  2. iszero: EQ = [D == 0] (ACT relu(1-1024D) / DVE is_equal), fp32 SBUF.
  3. Counting fused into layer-1 matmul: pre[f,j] = sum_i w1rep[i,f]*EQ[i,j]
     = w1[f]*count[j]  (W1REP = w1 replicated over 128 partitions).
  4. relu(pre + b1) drain, block-diag W2 matmul (both rows of a pair packed
     into 128 partitions), final drain adds 2*b2.
  5. Output written [row, f, n]-major; host transposes to [row, n, f].
"""
import sys

sys.path.insert(0, "/opt/trn_rl_repo")
import numpy as np

B, N, F = 4096, 128, 64
N_CORES = 8
BC = B // N_CORES       # rows per core
CHUNK = 64              # rows per input DMA chunk
S = 1.0 / 32.0

_BUILD_CACHE = {}


def build(reps=1):
    """Build (nc, tensor-name metadata) for the SPMD program. Cached."""
    if reps in _BUILD_CACHE:
        return _BUILD_CACHE[reps]
    from contextlib import ExitStack

    import concourse.tile as tile
    from concourse import bacc, mybir

    dt = mybir.dt
    AF = mybir.ActivationFunctionType
    OP = mybir.AluOpType

    nc = bacc.Bacc("TRN2", target_bir_lowering=False, debug=False,
                   num_devices=N_CORES)

    l5s = nc.dram_tensor("l5s", [BC, 5, N], dt.float16, kind="ExternalInput").ap()
    l5d = nc.dram_tensor("l5d", [BC, 5, N], dt.float16, kind="ExternalInput").ap()
    r5 = nc.dram_tensor("r5", [BC, 5, 2 * N], dt.float16, kind="ExternalInput").ap()
    w1rep_d = nc.dram_tensor("w1rep", [128, F], dt.float32, kind="ExternalInput").ap()
    w2t2_d = nc.dram_tensor("w2t2", [128, 128], dt.float32, kind="ExternalInput").ap()
    b1c_d = nc.dram_tensor("b1c", [128, 1], dt.float32, kind="ExternalInput").ap()
    b2c_d = nc.dram_tensor("b2c", [128, 1], dt.float32, kind="ExternalInput").ap()
    out_s = nc.dram_tensor("out_s", [BC, F, N], dt.float32, kind="ExternalOutput").ap()
    out_d = nc.dram_tensor("out_d", [BC, F, N], dt.float32, kind="ExternalOutput").ap()

    with tile.TileContext(nc) as tc, ExitStack() as ctx:
        consts = ctx.enter_context(tc.tile_pool(name="consts", bufs=1))
        pin = ctx.enter_context(tc.tile_pool(name="pin", bufs=2))
        peq = ctx.enter_context(tc.tile_pool(name="peq", bufs=4))
        ph = ctx.enter_context(tc.tile_pool(name="ph", bufs=3))
        po = ctx.enter_context(tc.tile_pool(name="po", bufs=3))
        psd = ctx.enter_context(tc.tile_pool(name="psd", bufs=2, space="PSUM"))
        psp = ctx.enter_context(tc.tile_pool(name="psp", bufs=2, space="PSUM"))
        pso = ctx.enter_context(tc.tile_pool(name="pso", bufs=2, space="PSUM"))

        w1rep = consts.tile([128, F], dt.float32)
        nc.sync.dma_start(w1rep[:], w1rep_d[:])
        w2t2 = consts.tile([128, 128], dt.float32)
        nc.sync.dma_start(w2t2[:], w2t2_d[:])
        b1c = consts.tile([128, 1], dt.float32)
        nc.sync.dma_start(b1c[:], b1c_d[:])
        b2c = consts.tile([128, 1], dt.float32)
        nc.sync.dma_start(b2c[:], b2c_d[:])

        def body():
            for ck in range(BC // CHUNK):
                r0 = ck * CHUNK
                Ls = pin.tile([5, CHUNK, N], dt.float16, tag="Ls")
                nc.sync.dma_start(
                    Ls[:], l5s[r0:r0 + CHUNK].rearrange("r p c -> p r c"))
                Ld = pin.tile([5, CHUNK, N], dt.float16, tag="Ld")
                nc.sync.dma_start(
                    Ld[:], l5d[r0:r0 + CHUNK].rearrange("r p c -> p r c"))
                Rc = pin.tile([5, CHUNK, 2 * N], dt.float16, tag="Rc")
                nc.sync.dma_start(
                    Rc[:], r5[r0:r0 + CHUNK].rearrange("r p c -> p r c"))

                for pr in range(CHUNK // 2):
                    gidx = ck * (CHUNK // 2) + pr
                    pre = psp.tile([128, 512], dt.float32, tag="pre")
                    for half, r in ((0, 2 * pr), (64, 2 * pr + 1)):
                        Dt = psd.tile([128, 512], dt.float32, tag="D")
                        nc.tensor.matmul(
                            Dt[:, 0:256],
                            Ls[:, r, :],
                            Rc[:, r, :],
                            start=True, stop=True)
                        nc.tensor.matmul(
                            Dt[:, 256:512],
                            Ld[:, r, :],
                            Rc[:, r, :],
                            start=True, stop=True)
                        EQ = peq.tile([128, 512], dt.float32, tag="EQ")
                        if r % 2 == 0:
                            nc.scalar.activation(EQ[:], Dt[:], AF.Relu,
                                                 bias=1.0, scale=-1024.0)
                        else:
                            nc.vector.tensor_scalar(EQ[:], Dt[:], 0.0, None,
                                                    OP.is_equal)
                        nc.tensor.matmul(pre[half:half + 64, 0:256],
                                         w1rep[:], EQ[:, 0:256],
                                         start=True, stop=True)
                        nc.tensor.matmul(pre[half:half + 64, 256:512],
                                         w1rep[:], EQ[:, 256:512],
                                         start=True, stop=True)
                    h = ph.tile([128, 512], dt.float32, tag="h")
                    if gidx % 2 == 0:
                        nc.scalar.activation(h[:], pre[:], AF.Relu,
                                             bias=b1c[:], scale=1.0)
                    else:
                        nc.vector.tensor_scalar(h[:], pre[:], b1c[:, 0:1], 0.0,
                                                OP.add, OP.max)
                    outp = pso.tile([128, 256], dt.float32, tag="outp")
                    nc.tensor.matmul(outp[:], w2t2[:], h[:, 0:256],
                                     start=True, stop=False)
                    nc.tensor.matmul(outp[:], w2t2[:], h[:, 256:512],
                                     start=False, stop=True)
                    osb = po.tile([128, 256], dt.float32, tag="osb")
                    if gidx % 2 == 0:
                        nc.vector.tensor_scalar(osb[:], outp[:], b2c[:, 0:1],
                                                None, OP.add)
                    else:
                        nc.scalar.activation(osb[:], outp[:], AF.Identity,
                                             bias=b2c[:], scale=1.0)
                    g0 = r0 + 2 * pr
                    nc.sync.dma_start(
                        out_s[g0:g0 + 2].rearrange("r g n -> (r g) n"),
                        osb[:, 0:128])
                    nc.sync.dma_start(
                        out_d[g0:g0 + 2].rearrange("r g n -> (r g) n"),
                        osb[:, 128:256])

        if reps == 1:
            body()
        else:
            with tc.For_i(0, reps, 1):
                body()

    nc.compile()
    _BUILD_CACHE[reps] = nc
    return nc


def _pack_inputs(src, dst, w1, b1, w2, b2):
    """Host-side packing: per-core in_maps for run_bass_kernel_spmd."""
    v = np.arange(1000, dtype=np.int64)
    sa_t = (v * S).astype(np.float16)
    q = v * v
    aaH_t = (((q >> 10) << 10) / 1024.0).astype(np.float16)
    aaL_t = ((q & 1023) / 1024.0).astype(np.float16)
    m2c_t = (-2.0 * v * S).astype(np.float16)

    src = np.asarray(src).astype(np.int64)
    dst = np.asarray(dst).astype(np.int64)
    cat = np.concatenate([src, dst], axis=1)          # [B, 256]
    ones_n = np.ones((B, N), dtype=np.float16)
    ones_c = np.ones((B, 2 * N), dtype=np.float16)

    l5s = np.stack([aaH_t[src], aaL_t[src], sa_t[src], ones_n, ones_n], axis=1)
    l5d = np.stack([aaH_t[dst], aaL_t[dst], sa_t[dst], ones_n, ones_n], axis=1)
    r5 = np.stack([ones_c, ones_c, m2c_t[cat], aaH_t[cat], aaL_t[cat]], axis=1)

    w1 = np.asarray(w1, dtype=np.float32).reshape(F)
    b1 = np.asarray(b1, dtype=np.float32).reshape(F)
    w2 = np.asarray(w2, dtype=np.float32)
    b2 = np.asarray(b2, dtype=np.float32).reshape(F)

    w1rep = np.tile(w1[None, :], (128, 1)).astype(np.float32)
    w2t2 = np.zeros((128, 128), dtype=np.float32)
    w2t2[:F, :F] = w2.T
    w2t2[F:, F:] = w2.T
    b1c = np.concatenate([b1, b1])[:, None].astype(np.float32)
    b2c = np.concatenate([2.0 * b2, 2.0 * b2])[:, None].astype(np.float32)

    in_maps = []
    for c in range(N_CORES):
        sl = slice(c * BC, (c + 1) * BC)
        in_maps.append({
            "l5s": np.ascontiguousarray(l5s[sl]),
            "l5d": np.ascontiguousarray(l5d[sl]),
            "r5": np.ascontiguousarray(r5[sl]),
            "w1rep": w1rep, "w2t2": w2t2, "b1c": b1c, "b2c": b2c,
        })
    return in_maps


def _unpack_outputs(results):
    src_feat = np.concatenate(
        [results[c]["out_s"].transpose(0, 2, 1) for c in range(N_CORES)], axis=0)
    dst_feat = np.concatenate(
        [results[c]["out_d"].transpose(0, 2, 1) for c in range(N_CORES)], axis=0)
    return np.ascontiguousarray(src_feat), np.ascontiguousarray(dst_feat)


def run_on_hw(in_maps, reps=1):
    from concourse.bass_utils import run_bass_kernel_spmd
    nc = build(reps)
    return run_bass_kernel_spmd(nc, in_maps, list(range(N_CORES)))


def kernel(src_neighbour_nodes_ids, dst_neighbour_nodes_ids, w1, b1, w2, b2):
    in_maps = _pack_inputs(src_neighbour_nodes_ids, dst_neighbour_nodes_ids,
                           w1, b1, w2, b2)
    res = run_on_hw(in_maps, reps=1)
    return _unpack_outputs(res.results)


# revision 7
# speedup vs baseline: 21.3171x; 21.3171x over previous
"""NeighborCooccurrenceEncoder Trainium2 kernel (8-core SPMD, data-parallel over B).

Algorithm per core (512 batch rows):
  1. Equality counting via PE matmul: D[i,j] = ((a_i - c_j)/32)^2 computed
     EXACTLY with fp16 operands (split-square trick) + fp32 PSUM accumulation.
     K=5 matmul: lhsT=(aaH, aaL, sa, 1, 1), rhs=(1, 1, -2sc, ccH, ccL).
  2. iszero: EQ = [D == 0] (ACT relu(1-1024D) / DVE is_equal), fp32 SBUF.
  3. Counting fused into layer-1 matmul: pre[f,j] = sum_i w1rep[i,f]*EQ[i,j]
     = w1[f]*count[j]  (W1REP = w1 replicated over 128 partitions).
  4. relu(pre + b1) drain, block-diag W2 matmul (both rows of a pair packed
     into 128 partitions), final drain adds 2*b2.
  5. Output written [row, f, n]-major; host transposes to [row, n, f].
"""
import sys

sys.path.insert(0, "/opt/trn_rl_repo")
import numpy as np

B, N, F = 4096, 128, 64
N_CORES = 8
BC = B // N_CORES       # rows per core
CHUNK = 64              # rows per input DMA chunk
S = 1.0 / 32.0

_BUILD_CACHE = {}


def build(reps=1, bench=False):
    """Build (nc, tensor-name metadata) for the SPMD program. Cached."""
    if (reps, bench) in _BUILD_CACHE:
        return _BUILD_CACHE[(reps, bench)]
    from contextlib import ExitStack

    import concourse.tile as tile
    from concourse import bacc, mybir

    dt = mybir.dt
    AF = mybir.ActivationFunctionType
    OP = mybir.AluOpType

    nc = bacc.Bacc("TRN2", target_bir_lowering=False, debug=False,
                   num_devices=N_CORES)

    l5s = nc.dram_tensor("l5s", [BC, 5, N], dt.float16, kind="ExternalInput").ap()
    l5d = nc.dram_tensor("l5d", [BC, 5, N], dt.float16, kind="ExternalInput").ap()
    r5 = nc.dram_tensor("r5", [BC, 5, 2 * N], dt.float16, kind="ExternalInput").ap()
    w1rep_d = nc.dram_tensor("w1rep", [128, F], dt.float32, kind="ExternalInput").ap()
    w2t2_d = nc.dram_tensor("w2t2", [128, 128], dt.float32, kind="ExternalInput").ap()
    b1c_d = nc.dram_tensor("b1c", [128, 1], dt.float32, kind="ExternalInput").ap()
    b2c_d = nc.dram_tensor("b2c", [128, 1], dt.float32, kind="ExternalInput").ap()
    if bench:
        out_s = nc.dram_tensor("out_s_i", [BC, F, N], dt.float32).ap()
        out_d = nc.dram_tensor("out_d_i", [BC, F, N], dt.float32).ap()
        probe = nc.dram_tensor("probe", [128, N], dt.float32,
                               kind="ExternalOutput").ap()
    else:
        out_s = nc.dram_tensor("out_s", [BC, F, N], dt.float32,
                               kind="ExternalOutput").ap()
        out_d = nc.dram_tensor("out_d", [BC, F, N], dt.float32,
                               kind="ExternalOutput").ap()

    with tile.TileContext(nc) as tc, ExitStack() as ctx:
        consts = ctx.enter_context(tc.tile_pool(name="consts", bufs=1))
        pin = ctx.enter_context(tc.tile_pool(name="pin", bufs=2))
        peq = ctx.enter_context(tc.tile_pool(name="peq", bufs=4))
        ph = ctx.enter_context(tc.tile_pool(name="ph", bufs=3))
        po = ctx.enter_context(tc.tile_pool(name="po", bufs=3))
        psd = ctx.enter_context(tc.tile_pool(name="psd", bufs=2, space="PSUM"))
        psp = ctx.enter_context(tc.tile_pool(name="psp", bufs=2, space="PSUM"))
        pso = ctx.enter_context(tc.tile_pool(name="pso", bufs=2, space="PSUM"))

        w1rep = consts.tile([128, F], dt.float32)
        nc.sync.dma_start(w1rep[:], w1rep_d[:])
        w2t2 = consts.tile([128, 128], dt.float32)
        nc.sync.dma_start(w2t2[:], w2t2_d[:])
        b1c = consts.tile([128, 1], dt.float32)
        nc.sync.dma_start(b1c[:], b1c_d[:])
        b2c = consts.tile([128, 1], dt.float32)
        nc.sync.dma_start(b2c[:], b2c_d[:])

        def body():
            for ck in range(BC // CHUNK):
                r0 = ck * CHUNK
                Ls = pin.tile([5, CHUNK, N], dt.float16, tag="Ls")
                nc.sync.dma_start(
                    Ls[:], l5s[r0:r0 + CHUNK].rearrange("r p c -> p r c"))
                Ld = pin.tile([5, CHUNK, N], dt.float16, tag="Ld")
                nc.sync.dma_start(
                    Ld[:], l5d[r0:r0 + CHUNK].rearrange("r p c -> p r c"))
                Rc = pin.tile([5, CHUNK, 2 * N], dt.float16, tag="Rc")
                nc.sync.dma_start(
                    Rc[:], r5[r0:r0 + CHUNK].rearrange("r p c -> p r c"))

                for pr in range(CHUNK // 2):
                    gidx = ck * (CHUNK // 2) + pr
                    pre = psp.tile([128, 512], dt.float32, tag="pre")
                    for half, r in ((0, 2 * pr), (64, 2 * pr + 1)):
                        Dt = psd.tile([128, 512], dt.float32, tag="D")
                        nc.tensor.matmul(
                            Dt[:, 0:256],
                            Ls[:, r, :],
                            Rc[:, r, :],
                            start=True, stop=True)
                        nc.tensor.matmul(
                            Dt[:, 256:512],
                            Ld[:, r, :],
                            Rc[:, r, :],
                            start=True, stop=True)
                        EQ = peq.tile([128, 512], dt.float32, tag="EQ")
                        if r % 2 == 0:
                            nc.scalar.activation(EQ[:], Dt[:], AF.Relu,
                                                 bias=1.0, scale=-1024.0)
                        else:
                            nc.vector.tensor_scalar(EQ[:], Dt[:], 0.0, None,
                                                    OP.is_equal)
                        nc.tensor.matmul(pre[half:half + 64, 0:256],
                                         w1rep[:], EQ[:, 0:256],
                                         start=True, stop=True)
                        nc.tensor.matmul(pre[half:half + 64, 256:512],
                                         w1rep[:], EQ[:, 256:512],
                                         start=True, stop=True)
                    h = ph.tile([128, 512], dt.float32, tag="h")
                    if gidx % 2 == 0:
                        nc.scalar.activation(h[:], pre[:], AF.Relu,
                                             bias=b1c[:], scale=1.0)
                    else:
                        nc.vector.tensor_scalar(h[:], pre[:], b1c[:, 0:1], 0.0,
                                                OP.add, OP.max)
                    outp = pso.tile([128, 256], dt.float32, tag="outp")
                    nc.tensor.matmul(outp[:], w2t2[:], h[:, 0:256],
                                     start=True, stop=False)
                    nc.tensor.matmul(outp[:], w2t2[:], h[:, 256:512],
                                     start=False, stop=True)
                    osb = po.tile([128, 256], dt.float32, tag="osb")
                    if gidx % 2 == 0:
                        nc.vector.tensor_scalar(osb[:], outp[:], b2c[:, 0:1],
                                                None, OP.add)
                    else:
                        nc.scalar.activation(osb[:], outp[:], AF.Identity,
                                             bias=b2c[:], scale=1.0)
                    g0 = r0 + 2 * pr
                    nc.sync.dma_start(
                        out_s[g0:g0 + 2].rearrange("r g n -> (r g) n"),
                        osb[:, 0:128])
                    nc.sync.dma_start(
                        out_d[g0:g0 + 2].rearrange("r g n -> (r g) n"),
                        osb[:, 128:256])

        if reps == 1:
            body()
        else:
            with tc.For_i(0, reps, 1):
                body()
        if bench:
            pb = po.tile([128, N], dt.float32, tag="pb")
            nc.sync.dma_start(pb[:], out_s[0:2].rearrange("r g n -> (r g) n"))
            nc.sync.dma_start(probe[:], pb[:])

    nc.compile()
    _BUILD_CACHE[(reps, bench)] = nc
    return nc


def _pack_inputs(src, dst, w1, b1, w2, b2):
    """Host-side packing: per-core in_maps for run_bass_kernel_spmd."""
    v = np.arange(1000, dtype=np.int64)
    sa_t = (v * S).astype(np.float16)
    q = v * v
    aaH_t = (((q >> 10) << 10) / 1024.0).astype(np.float16)
    aaL_t = ((q & 1023) / 1024.0).astype(np.float16)
    m2c_t = (-2.0 * v * S).astype(np.float16)

    src = np.asarray(src).astype(np.int64)
    dst = np.asarray(dst).astype(np.int64)
    cat = np.concatenate([src, dst], axis=1)          # [B, 256]
    ones_n = np.ones((B, N), dtype=np.float16)
    ones_c = np.ones((B, 2 * N), dtype=np.float16)

    l5s = np.stack([aaH_t[src], aaL_t[src], sa_t[src], ones_n, ones_n], axis=1)
    l5d = np.stack([aaH_t[dst], aaL_t[dst], sa_t[dst], ones_n, ones_n], axis=1)
    r5 = np.stack([ones_c, ones_c, m2c_t[cat], aaH_t[cat], aaL_t[cat]], axis=1)

    w1 = np.asarray(w1, dtype=np.float32).reshape(F)
    b1 = np.asarray(b1, dtype=np.float32).reshape(F)
    w2 = np.asarray(w2, dtype=np.float32)
    b2 = np.asarray(b2, dtype=np.float32).reshape(F)

    w1rep = np.tile(w1[None, :], (128, 1)).astype(np.float32)
    w2t2 = np.zeros((128, 128), dtype=np.float32)
    w2t2[:F, :F] = w2.T
    w2t2[F:, F:] = w2.T
    b1c = np.concatenate([b1, b1])[:, None].astype(np.float32)
    b2c = np.concatenate([2.0 * b2, 2.0 * b2])[:, None].astype(np.float32)

    in_maps = []
    for c in range(N_CORES):
        sl = slice(c * BC, (c + 1) * BC)
        in_maps.append({
            "l5s": np.ascontiguousarray(l5s[sl]),
            "l5d": np.ascontiguousarray(l5d[sl]),
            "r5": np.ascontiguousarray(r5[sl]),
            "w1rep": w1rep, "w2t2": w2t2, "b1c": b1c, "b2c": b2c,
        })
    return in_maps


def _unpack_outputs(results):
    src_feat = np.concatenate(
        [results[c]["out_s"].transpose(0, 2, 1) for c in range(N_CORES)], axis=0)
    dst_feat = np.concatenate(
        [results[c]["out_d"].transpose(0, 2, 1) for c in range(N_CORES)], axis=0)
    return np.ascontiguousarray(src_feat), np.ascontiguousarray(dst_feat)


def run_on_hw(in_maps, reps=1, bench=False):
    from concourse.bass_utils import run_bass_kernel_spmd
    nc = build(reps, bench)
    return run_bass_kernel_spmd(nc, in_maps, list(range(N_CORES)))


def kernel(src_neighbour_nodes_ids, dst_neighbour_nodes_ids, w1, b1, w2, b2):
    in_maps = _pack_inputs(src_neighbour_nodes_ids, dst_neighbour_nodes_ids,
                           w1, b1, w2, b2)
    res = run_on_hw(in_maps, reps=1)
    return _unpack_outputs(res.results)


# revision 12
# speedup vs baseline: 64.7124x; 3.0357x over previous
"""NeighborCooccurrenceEncoder Trainium2 kernel (8-core SPMD, data-parallel over B).

Algorithm per core (512 batch rows):
  1. Equality counting via PE matmul: D[i,j] = ((a_i - c_j)/32)^2 computed
     EXACTLY with fp16 operands (split-square trick) + fp32 PSUM accumulation.
     K=5 matmul: lhsT=(aaH, aaL, sa, 1, 1), rhs=(1, 1, -2sc, ccH, ccL).
  2. iszero: EQ = [D == 0] (ACT relu(1-1024D) / DVE is_equal), fp32 SBUF.
  3. Counting fused into layer-1 matmul: pre[f,j] = sum_i w1rep[i,f]*EQ[i,j]
     = w1[f]*count[j]  (W1REP = w1 replicated over 128 partitions).
  4. relu(pre + b1) drain, block-diag W2 matmul (both rows of a pair packed
     into 128 partitions), final drain adds 2*b2.
  5. Output written [row, f, n]-major; host transposes to [row, n, f].
"""
import sys

sys.path.insert(0, "/opt/trn_rl_repo")
import numpy as np

B, N, F = 4096, 128, 64
N_CORES = 8
BC = B // N_CORES       # rows per core
CHUNK = 64              # rows per input DMA chunk
S = 1.0 / 32.0

_BUILD_CACHE = {}


def build(reps=1, bench=False):
    """Build (nc, tensor-name metadata) for the SPMD program. Cached."""
    if (reps, bench) in _BUILD_CACHE:
        return _BUILD_CACHE[(reps, bench)]
    from contextlib import ExitStack

    import concourse.tile as tile
    from concourse import bacc, mybir

    dt = mybir.dt
    AF = mybir.ActivationFunctionType
    OP = mybir.AluOpType

    nc = bacc.Bacc("TRN2", target_bir_lowering=False, debug=False,
                   num_devices=N_CORES)

    l5s = nc.dram_tensor("l5s", [BC, 5, N], dt.float16, kind="ExternalInput").ap()
    l5d = nc.dram_tensor("l5d", [BC, 5, N], dt.float16, kind="ExternalInput").ap()
    r5 = nc.dram_tensor("r5", [BC, 5, 2 * N], dt.float16, kind="ExternalInput").ap()
    w1rep_d = nc.dram_tensor("w1rep", [128, F], dt.bfloat16, kind="ExternalInput").ap()
    w2t2_d = nc.dram_tensor("w2t2", [128, 128], dt.bfloat16, kind="ExternalInput").ap()
    b1c_d = nc.dram_tensor("b1c", [128, 1], dt.float32, kind="ExternalInput").ap()
    b2c_d = nc.dram_tensor("b2c", [128, 1], dt.float32, kind="ExternalInput").ap()
    if bench:
        out_s = nc.dram_tensor("out_s_i", [BC, F, N], dt.float32).ap()
        out_d = nc.dram_tensor("out_d_i", [BC, F, N], dt.float32).ap()
        probe = nc.dram_tensor("probe", [128, N], dt.float32,
                               kind="ExternalOutput").ap()
    else:
        out_s = nc.dram_tensor("out_s", [BC, F, N], dt.float32,
                               kind="ExternalOutput").ap()
        out_d = nc.dram_tensor("out_d", [BC, F, N], dt.float32,
                               kind="ExternalOutput").ap()

    with tile.TileContext(nc) as tc, ExitStack() as ctx:
        consts = ctx.enter_context(tc.tile_pool(name="consts", bufs=1))
        pin = ctx.enter_context(tc.tile_pool(name="pin", bufs=2))
        peq = ctx.enter_context(tc.tile_pool(name="peq", bufs=4))
        ph = ctx.enter_context(tc.tile_pool(name="ph", bufs=3))
        po = ctx.enter_context(tc.tile_pool(name="po", bufs=3))
        psd = ctx.enter_context(tc.tile_pool(name="psd", bufs=3, space="PSUM"))
        psp = ctx.enter_context(tc.tile_pool(name="psp", bufs=2, space="PSUM"))
        pso = ctx.enter_context(tc.tile_pool(name="pso", bufs=2, space="PSUM"))

        w1rep = consts.tile([128, F], dt.bfloat16)
        nc.sync.dma_start(w1rep[:], w1rep_d[:])
        w2t2 = consts.tile([128, 128], dt.bfloat16)
        nc.sync.dma_start(w2t2[:], w2t2_d[:])
        b1c = consts.tile([128, 1], dt.float32)
        nc.sync.dma_start(b1c[:], b1c_d[:])
        b2c = consts.tile([128, 1], dt.float32)
        nc.sync.dma_start(b2c[:], b2c_d[:])

        def body():
            for ck in range(BC // CHUNK):
                r0 = ck * CHUNK
                Ls = pin.tile([5, CHUNK, N], dt.float16, tag="Ls")
                nc.sync.dma_start(
                    Ls[:], l5s[r0:r0 + CHUNK].rearrange("r p c -> p r c"))
                Ld = pin.tile([5, CHUNK, N], dt.float16, tag="Ld")
                nc.sync.dma_start(
                    Ld[:], l5d[r0:r0 + CHUNK].rearrange("r p c -> p r c"))
                Rc = pin.tile([5, CHUNK, 2 * N], dt.float16, tag="Rc")
                nc.sync.dma_start(
                    Rc[:], r5[r0:r0 + CHUNK].rearrange("r p c -> p r c"))

                for grp in range(CHUNK // 8):
                    osb = po.tile([128, 1024], dt.float32, tag="osb")
                    for sub in range(4):          # 4 pairs per group
                        pr = 4 * grp + sub
                        gidx = ck * (CHUNK // 2) + pr
                        pre = psp.tile([128, 512], dt.float32, tag="pre")
                        for half, r in ((0, 2 * pr), (64, 2 * pr + 1)):
                            Dt = psd.tile([128, 512], dt.float32, tag="D")
                            nc.tensor.matmul(
                                Dt[:, 0:256], Ls[:, r, :], Rc[:, r, :],
                                start=True, stop=True)
                            nc.tensor.matmul(
                                Dt[:, 256:512], Ld[:, r, :], Rc[:, r, :],
                                start=True, stop=True)
                            EQ = peq.tile([128, 512], dt.bfloat16, tag="EQ")
                            if r % 2 == 0:
                                nc.scalar.activation(EQ[:], Dt[:], AF.Relu,
                                                     bias=1.0, scale=-1024.0)
                            else:
                                nc.vector.tensor_scalar(EQ[:], Dt[:], 0.0,
                                                        None, OP.is_equal)
                            nc.tensor.matmul(
                                pre[half:half + 64, 0:512],
                                w1rep[:], EQ[:, 0:512],
                                start=True, stop=True)
                        h = ph.tile([128, 512], dt.bfloat16, tag="h")
                        nc.scalar.activation(h[:], pre[:], AF.Relu,
                                             bias=b1c[:], scale=1.0)
                        outp = pso.tile([128, 256], dt.float32, tag="outp")
                        nc.tensor.matmul(outp[:], w2t2[:], h[:, 0:256],
                                         start=True, stop=False)
                        nc.tensor.matmul(outp[:], w2t2[:], h[:, 256:512],
                                         start=False, stop=True)
                        nc.vector.tensor_scalar(
                            osb[:, sub * 256:sub * 256 + 256], outp[:],
                            b2c[:, 0:1], None, OP.add)
                    g0 = r0 + 8 * grp
                    osb4 = osb[:].rearrange("p (pair x n) -> p pair x n",
                                            pair=4, x=2)
                    nc.sync.dma_start(
                        out_s[g0:g0 + 8].rearrange(
                            "(pair rp) g n -> (rp g) pair n", rp=2),
                        osb4[:, :, 0, :])
                    nc.sync.dma_start(
                        out_d[g0:g0 + 8].rearrange(
                            "(pair rp) g n -> (rp g) pair n", rp=2),
                        osb4[:, :, 1, :])

        if reps == 1:
            body()
        else:
            with tc.For_i(0, reps, 1):
                body()
        if bench:
            pb = po.tile([128, N], dt.float32, tag="pb")
            nc.sync.dma_start(pb[:], out_s[0:2].rearrange("r g n -> (r g) n"))
            nc.sync.dma_start(probe[:], pb[:])

    nc.compile()
    _BUILD_CACHE[(reps, bench)] = nc
    return nc


def _pack_inputs(src, dst, w1, b1, w2, b2):
    """Host-side packing: per-core in_maps for run_bass_kernel_spmd."""
    v = np.arange(1000, dtype=np.int64)
    sa_t = (v * S).astype(np.float16)
    q = v * v
    aaH_t = (((q >> 10) << 10) / 1024.0).astype(np.float16)
    aaL_t = ((q & 1023) / 1024.0).astype(np.float16)
    m2c_t = (-2.0 * v * S).astype(np.float16)

    src = np.asarray(src).astype(np.int64)
    dst = np.asarray(dst).astype(np.int64)
    cat = np.concatenate([src, dst], axis=1)          # [B, 256]
    ones_n = np.ones((B, N), dtype=np.float16)
    ones_c = np.ones((B, 2 * N), dtype=np.float16)

    l5s = np.stack([aaH_t[src], aaL_t[src], sa_t[src], ones_n, ones_n], axis=1)
    l5d = np.stack([aaH_t[dst], aaL_t[dst], sa_t[dst], ones_n, ones_n], axis=1)
    r5 = np.stack([ones_c, ones_c, m2c_t[cat], aaH_t[cat], aaL_t[cat]], axis=1)

    w1 = np.asarray(w1, dtype=np.float32).reshape(F)
    b1 = np.asarray(b1, dtype=np.float32).reshape(F)
    w2 = np.asarray(w2, dtype=np.float32)
    b2 = np.asarray(b2, dtype=np.float32).reshape(F)

    import ml_dtypes
    w1rep = np.tile(w1[None, :], (128, 1)).astype(ml_dtypes.bfloat16)
    w2t2 = np.zeros((128, 128), dtype=np.float32)
    w2t2[:F, :F] = w2.T
    w2t2[F:, F:] = w2.T
    w2t2 = w2t2.astype(ml_dtypes.bfloat16)
    b1c = np.concatenate([b1, b1])[:, None].astype(np.float32)
    b2c = np.concatenate([2.0 * b2, 2.0 * b2])[:, None].astype(np.float32)

    in_maps = []
    for c in range(N_CORES):
        sl = slice(c * BC, (c + 1) * BC)
        in_maps.append({
            "l5s": np.ascontiguousarray(l5s[sl]),
            "l5d": np.ascontiguousarray(l5d[sl]),
            "r5": np.ascontiguousarray(r5[sl]),
            "w1rep": w1rep, "w2t2": w2t2, "b1c": b1c, "b2c": b2c,
        })
    return in_maps


def _unpack_outputs(results):
    src_feat = np.concatenate(
        [results[c]["out_s"].transpose(0, 2, 1) for c in range(N_CORES)], axis=0)
    dst_feat = np.concatenate(
        [results[c]["out_d"].transpose(0, 2, 1) for c in range(N_CORES)], axis=0)
    return np.ascontiguousarray(src_feat), np.ascontiguousarray(dst_feat)


def run_on_hw(in_maps, reps=1, bench=False):
    from concourse.bass_utils import run_bass_kernel_spmd
    nc = build(reps, bench)
    return run_bass_kernel_spmd(nc, in_maps, list(range(N_CORES)))


def kernel(src_neighbour_nodes_ids, dst_neighbour_nodes_ids, w1, b1, w2, b2):
    in_maps = _pack_inputs(src_neighbour_nodes_ids, dst_neighbour_nodes_ids,
                           w1, b1, w2, b2)
    res = run_on_hw(in_maps, reps=1)
    return _unpack_outputs(res.results)
